# revision 13
# baseline (speedup 1.0000x reference)
"""Causal self-attention TRN2 kernel (8 NeuronCores).

Problem: x[4,2048,1024] f32, w_qkv[3072,1024], w_proj[1024,1024]
  qkv = x @ w_qkv.T; per-head causal softmax(q k^T / sqrt(64)) v; out @ w_proj.T

Sharding: 8 cores = (head-group hg in {0,1}) x (batch b in {0..3}).
  Core computes its 8 heads for its batch; partial y (contracted over its
  512 channels of w_proj input dim) is summed pairwise on host.

Per-core dataflow (all matmul inputs float32r = full-rate TF32-like):
  Stage 1: q,k projections -> qkT [1024, 2048] (f on partitions).
  Stage 2 (fused, per t-chunk): v projection -> V [2048, 8x65] (natural;
    col 65h+64 = ones giving the softmax denominator for free in PV),
    then attention for that i-block:
      S^T tiles [j=128, i=2x512] via PE (K=64, even/odd head row-tiled so
      the two MMs run concurrently), exp via ACT (scale=1/8) psum->sbuf,
      causal mask on straddling tiles via gpsimd.affine_select,
      PV via PE lhsT=[V|1] -> psum [65,512] (row 64 = denominator),
      normalize: DVE recip-approx + gpsimd partition_broadcast + DVE mul
      -> attnT [512, 2048]  (c_local on partitions)
    projection (attnT.T @ w_projT) interleaved lazily as PE filler.
"""

import numpy as np

import concourse.bacc as bacc
import concourse.mybir as mybir
import concourse.tile as tile
from concourse.bass_utils import run_bass_kernel_spmd

F32 = mybir.dt.float32
F32R = mybir.dt.float32r
EXP = mybir.ActivationFunctionType.Exp

B, T, C = 4, 2048, 1024
NH, HD = 16, 64
HPC = 8                      # heads per core
FH = HPC * HD                # 512: per-core q/k/v feature width
NCORES = 8
LAG = 2                      # scores->PV software-pipeline depth (j-tiles)

_CACHE = {}


def build_nc():
    nc = bacc.Bacc()
    xT_d = nc.dram_tensor("xT", [C, T], F32R, kind="ExternalInput")
    wqkvT_d = nc.dram_tensor("wqkvT", [C, 3 * FH], F32R, kind="ExternalInput")
    wprojT_d = nc.dram_tensor("wprojT", [FH, C], F32R, kind="ExternalInput")
    y_d = nc.dram_tensor("y", [T, C], F32, kind="ExternalOutput")

    NKT = C // 128           # 8 c-tiles (contraction for qkv)
    NTT = T // 128           # 16 t-tiles
    NTC = T // 512           # 4 t-chunks / i-blocks

    with tile.TileContext(nc) as tc:
        with (
            # ---------------- persistent pools (whole kernel) --------------
            tc.tile_pool(name="qkt", bufs=1) as qkt_pool,
            tc.tile_pool(name="vp", bufs=1) as v_pool,
            tc.tile_pool(name="wproj", bufs=1) as wproj_pool,
        ):
            qkT = [qkt_pool.tile([128, T], F32R, tag=f"qkt{i}", name=f"qkt{i}")
                   for i in range(8)]
            v_sb = [v_pool.tile([128, HPC * 65], F32R, tag=f"v{i}", name=f"v{i}")
                    for i in range(NTT)]
            wprojT = [wproj_pool.tile([128, C], F32R, tag=f"wp{i}", name=f"wp{i}")
                      for i in range(4)]
            for g in range(4):
                nc.sync.dma_start(out=wprojT[g][:],
                                  in_=wprojT_d[g * 128:(g + 1) * 128, :])

            # ---------------- stage 1: q,k projections ---------------------
            with (
                tc.tile_pool(name="wqk", bufs=1) as wqk_pool,
                tc.tile_pool(name="xc1", bufs=1) as x1_pool,
                tc.tile_pool(name="ps1", bufs=1, space="PSUM") as ps1,
            ):
                wqk = [wqk_pool.tile([128, 2 * FH], F32R, tag=f"wqk{k}",
                                     name=f"wqk{k}") for k in range(NKT)]
                for k in range(NKT):
                    nc.sync.dma_start(out=wqk[k][:],
                                      in_=wqkvT_d[k * 128:(k + 1) * 128,
                                                  0:2 * FH])
                for tcb in range(NTC):
                    xc = [x1_pool.tile([128, 512], F32R, tag=f"xc{k}", bufs=2,
                                       name=f"xc{tcb}_{k}")
                          for k in range(NKT)]
                    for k in range(NKT):
                        nc.sync.dma_start(
                            out=xc[k][:],
                            in_=xT_d[k * 128:(k + 1) * 128,
                                     tcb * 512:(tcb + 1) * 512])
                    for fi in range(8):      # 0-3 q rows, 4-7 k rows
                        ps = ps1.tile([128, 512], F32, tag="ps1", bufs=6,
                                      name=f"psqk{tcb}_{fi}")
                        for k in range(NKT):
                            nc.tensor.matmul(ps[:],
                                             wqk[k][:, fi * 128:(fi + 1) * 128],
                                             xc[k][:],
                                             start=(k == 0), stop=(k == NKT - 1))
                        nc.vector.tensor_copy(
                            out=qkT[fi][:, tcb * 512:(tcb + 1) * 512], in_=ps[:])

            # ------- stage 2: v projection + attention + output proj -------
            with (
                tc.tile_pool(name="wv", bufs=1) as wv_pool,
                tc.tile_pool(name="xc2", bufs=1) as x2_pool,
                tc.tile_pool(name="attnt", bufs=1) as attnt_pool,
                tc.tile_pool(name="pt", bufs=1) as pt_pool,
                tc.tile_pool(name="stage", bufs=1) as stage_pool,
                tc.tile_pool(name="nrm", bufs=1) as nrm_pool,
                tc.tile_pool(name="psM", bufs=1, space="PSUM") as psM,
                tc.tile_pool(name="psS", bufs=1, space="PSUM") as psS,
                tc.tile_pool(name="psPV", bufs=1, space="PSUM") as psPV,
            ):
                wv = [wv_pool.tile([128, FH], F32R, tag=f"wv{k}",
                                   name=f"wv{k}") for k in range(NKT)]
                for k in range(NKT):
                    nc.sync.dma_start(out=wv[k][:],
                                      in_=wqkvT_d[k * 128:(k + 1) * 128,
                                                  2 * FH:3 * FH])
                attnT = [attnt_pool.tile([128, T], F32R, tag=f"at{g}",
                                         name=f"at{g}") for g in range(4)]

                def emit_proj(ti, fc):
                    po = psM.tile([128, 512], F32, tag="mm512", bufs=2,
                                  name=f"po{ti}_{fc}")
                    for g in range(4):
                        nc.tensor.matmul(
                            po[:],
                            attnT[g][:, ti * 128:(ti + 1) * 128],
                            wprojT[g][:, fc * 512:(fc + 1) * 512],
                            start=(g == 0), stop=(g == 3))
                    ot = stage_pool.tile([128, 512], F32, tag="ot",
                                         bufs=2, name=f"ot{ti}_{fc}")
                    nc.vector.tensor_copy(out=ot[:], in_=po[:])
                    nc.sync.dma_start(
                        out=y_d[ti * 128:(ti + 1) * 128,
                                fc * 512:(fc + 1) * 512],
                        in_=ot[:])

                pending = []
                for tcb in range(NTC):
                    # ---- v projection for this t-chunk ----
                    for tl in range(4):
                        ti = tcb * 4 + tl
                        xc = [x2_pool.tile([128, 128], F32R, tag=f"x2_{k}",
                                           bufs=2, name=f"x2_{ti}_{k}")
                              for k in range(NKT)]
                        for k in range(NKT):
                            nc.sync.dma_start(
                                out=xc[k][:],
                                in_=xT_d[k * 128:(k + 1) * 128,
                                         ti * 128:(ti + 1) * 128])
                        ps = psM.tile([128, 512], F32, tag="mm512", bufs=2,
                                      name=f"psv{ti}")
                        for k in range(NKT):
                            nc.tensor.matmul(ps[:],
                                             xc[k][:],
                                             wv[k][:],
                                             start=(k == 0), stop=(k == NKT - 1))
                        vt = v_sb[ti]
                        vv = vt[:].rearrange("p (h x) -> p h x", h=HPC)
                        nc.vector.memset(vt[:].bitcast(F32), 1.0)
                        nc.vector.tensor_copy(
                            out=vv[:, :, 0:64],
                            in_=ps[:].rearrange("p (h x) -> p h x", h=HPC))

                    # ---- attention for i-block bi = tcb ----
                    bi = tcb
                    njt = 4 * bi + 4
                    for hp in range(4):          # head pair (2hp, 2hp+1)
                        for _ in range(2):
                            if pending:
                                emit_proj(*pending.pop(0))
                        qt = qkT[hp]
                        kt = qkT[4 + hp]
                        pts = []
                        pvs = [psPV.tile([65, 512], F32, tag="pv", bufs=2,
                                         name=f"pv{bi}_{hp}_{par}")
                               for par in range(2)]

                        def emit_scores(jj, bi=bi, hp=hp, qt=qt, kt=kt,
                                        pts=pts):
                            sps = psS.tile([128, 1024], F32, tag="sps", bufs=2,
                                           name=f"sps{bi}_{hp}_{jj}")
                            # even head on PE rows 0-63, odd head on rows
                            # 64-127 -> the two MMs run concurrently
                            for par in range(2):
                                off = par * 64
                                nc.tensor.matmul(
                                    sps[:, par * 512:(par + 1) * 512],
                                    kt[off:off + 64, jj * 128:(jj + 1) * 128],
                                    qt[off:off + 64, bi * 512:(bi + 1) * 512],
                                    start=True, stop=True)
                            pt = pt_pool.tile([128, 1024], F32R, tag="pt",
                                              bufs=LAG + 2,
                                              name=f"pt{bi}_{hp}_{jj}")
                            if jj < 4 * bi:
                                nc.scalar.activation(pt[:], sps[:], EXP,
                                                     scale=0.125)
                            else:
                                st = stage_pool.tile([128, 1024], F32R,
                                                     tag="st", bufs=2,
                                                     name=f"st{bi}_{hp}_{jj}")
                                nc.scalar.activation(st[:], sps[:], EXP,
                                                     scale=0.125)
                                r0 = jj - 4 * bi
                                nc.gpsimd.affine_select(
                                    out=pt[:].rearrange("p (b i) -> p b i", b=2),
                                    in_=st[:].rearrange("p (b i) -> p b i", b=2),
                                    compare_op=mybir.AluOpType.is_ge,
                                    fill=0.0,
                                    base=-128 * r0,
                                    pattern=[[0, 2], [1, 512]],
                                    channel_multiplier=-1,
                                )
                            pts.append(pt)

                        def emit_pv(jj, bi=bi, hp=hp, pts=pts, pvs=pvs,
                                    njt=njt):
                            for par in range(2):
                                h = 2 * hp + par
                                nc.tensor.matmul(
                                    pvs[par][:],
                                    v_sb[jj][:, h * 65:h * 65 + 65],
                                    pts[jj][:, par * 512:(par + 1) * 512],
                                    start=(jj == 0), stop=(jj == njt - 1))

                        # software pipeline: PV lags scores by LAG j-tiles
                        for jj in range(njt):
                            emit_scores(jj)
                            if jj >= LAG:
                                emit_pv(jj - LAG)
                        for jj in range(max(0, njt - LAG), njt):
                            emit_pv(jj)
                        for par in range(2):
                            pv = pvs[par]
                            den = nrm_pool.tile([1, 512], F32, tag="den",
                                                bufs=1, name=f"den{bi}_{hp}_{par}")
                            nc.vector.tensor_copy(out=den[0:1, :],
                                                  in_=pv[64:65, :])
                            rec = nrm_pool.tile([1, 512], F32, tag="rec",
                                                bufs=1, name=f"rec{bi}_{hp}_{par}")
                            nc.vector.reciprocal_approx_fast(
                                out=rec[0:1, :], in_=den[0:1, :])
                            bc = nrm_pool.tile([64, 512], F32, tag="bc",
                                               bufs=2, name=f"bc{bi}_{hp}_{par}")
                            nc.gpsimd.partition_broadcast(bc[:, :], rec[0:1, :])
                            nc.vector.tensor_mul(
                                out=attnT[hp][par * 64:par * 64 + 64,
                                              bi * 512:(bi + 1) * 512],
                                in0=pv[0:64, :], in1=bc[:, :])
                    # queue this i-block's projection groups
                    for tl in range(4):
                        for fc in range(2):
                            pending.append((bi * 4 + tl, fc))
                for pf in pending:
                    emit_proj(*pf)
    nc.compile()
    return nc


def _get_nc():
    if "nc" not in _CACHE:
        _CACHE["nc"] = build_nc()
    return _CACHE["nc"]


def kernel(x, w_qkv, w_proj, _trace=False):
    x = np.asarray(x, dtype=np.float32)
    w_qkv = np.asarray(w_qkv, dtype=np.float32)
    w_proj = np.asarray(w_proj, dtype=np.float32)

    nc = _get_nc()
    in_maps = []
    for c in range(NCORES):
        hg, b = c // 4, c % 4
        xT = np.ascontiguousarray(x[b].T)                       # [1024, 2048]
        rows = []
        for sec in range(3):                                     # q, k, v
            rows.append(w_qkv[sec * C + hg * FH: sec * C + (hg + 1) * FH])
        wqkvT = np.ascontiguousarray(np.concatenate(rows, 0).T)  # [1024, 1536]
        wprojT = np.ascontiguousarray(w_proj[:, hg * FH:(hg + 1) * FH].T)
        in_maps.append({"xT": xT, "wqkvT": wqkvT, "wprojT": wprojT})

    res = run_bass_kernel_spmd(nc, in_maps, list(range(NCORES)), trace=_trace)
    if _trace:
        _CACHE["exec_time_ns"] = res.exec_time_ns

    y = np.empty((B, T, C), dtype=np.float32)
    for b in range(B):
        y[b] = res.results[b]["y"] + res.results[4 + b]["y"]
    return y


# revision 15
# speedup vs baseline: 1.0006x; 1.0006x over previous
"""Causal self-attention TRN2 kernel (8 NeuronCores).

Problem: x[4,2048,1024] f32, w_qkv[3072,1024], w_proj[1024,1024]
  qkv = x @ w_qkv.T; per-head causal softmax(q k^T / sqrt(64)) v; out @ w_proj.T

Sharding: 8 cores = (head-group hg in {0,1}) x (batch b in {0..3}).
  Core computes its 8 heads for its batch; partial y (contracted over its
  512 channels of w_proj input dim) is summed pairwise on host.

Per-core dataflow (all matmul inputs float32r = full-rate TF32-like):
  Stage 1: q,k projections -> qkT [1024, 2048] (f on partitions).
  Stage 2 (fused, per t-chunk): v projection -> V [2048, 8x65] (natural;
    col 65h+64 = ones giving the softmax denominator for free in PV),
    then attention for that i-block:
      S^T tiles [j=128, i=2x512] via PE (K=64, even/odd head row-tiled so
      the two MMs run concurrently), exp via ACT (scale=1/8) psum->sbuf,
      causal mask on straddling tiles via gpsimd.affine_select,
      PV via PE lhsT=[V|1] -> psum [65,512] (row 64 = denominator),
      normalize: DVE recip-approx + gpsimd partition_broadcast + DVE mul
      -> attnT [512, 2048]  (c_local on partitions)
    projection (attnT.T @ w_projT) interleaved lazily as PE filler.
"""

import numpy as np

import concourse.bacc as bacc
import concourse.mybir as mybir
import concourse.tile as tile
from concourse.bass_utils import run_bass_kernel_spmd

F32 = mybir.dt.float32
F32R = mybir.dt.float32r
EXP = mybir.ActivationFunctionType.Exp

B, T, C = 4, 2048, 1024
NH, HD = 16, 64
HPC = 8                      # heads per core
FH = HPC * HD                # 512: per-core q/k/v feature width
NCORES = 8
LAG = 2                      # scores->PV software-pipeline depth (j-tiles)

_CACHE = {}


def build_nc():
    nc = bacc.Bacc()
    xT_d = nc.dram_tensor("xT", [C, T], F32R, kind="ExternalInput")
    wqkvT_d = nc.dram_tensor("wqkvT", [C, 3 * FH], F32R, kind="ExternalInput")
    wprojT_d = nc.dram_tensor("wprojT", [FH, C], F32R, kind="ExternalInput")
    y_d = nc.dram_tensor("y", [T, C], F32, kind="ExternalOutput")

    NKT = C // 128           # 8 c-tiles (contraction for qkv)
    NTT = T // 128           # 16 t-tiles
    NTC = T // 512           # 4 t-chunks / i-blocks

    with tile.TileContext(nc) as tc:
        with (
            # ---------------- persistent pools (whole kernel) --------------
            tc.tile_pool(name="qkt", bufs=1) as qkt_pool,
            tc.tile_pool(name="vp", bufs=1) as v_pool,
            tc.tile_pool(name="wproj", bufs=1) as wproj_pool,
        ):
            qkT = [qkt_pool.tile([128, T], F32R, tag=f"qkt{i}", name=f"qkt{i}")
                   for i in range(8)]
            v_sb = [v_pool.tile([128, HPC * 65], F32R, tag=f"v{i}", name=f"v{i}")
                    for i in range(NTT)]
            wprojT = [wproj_pool.tile([128, C], F32R, tag=f"wp{i}", name=f"wp{i}")
                      for i in range(4)]

            wv_pool_outer = tc.tile_pool(name="wv", bufs=1)
            wv_pool = wv_pool_outer.__enter__()
            wv = [wv_pool.tile([128, FH], F32R, tag=f"wv{k}",
                               name=f"wv{k}") for k in range(NKT)]

            # ---------------- stage 1: q,k projections ---------------------
            with (
                tc.tile_pool(name="wqk", bufs=1) as wqk_pool,
                tc.tile_pool(name="xc1", bufs=1) as x1_pool,
                tc.tile_pool(name="ps1", bufs=1, space="PSUM") as ps1,
            ):
                wqk = [wqk_pool.tile([128, 2 * FH], F32R, tag=f"wqk{k}",
                                     name=f"wqk{k}") for k in range(NKT)]
                for tcb in range(NTC):
                    xc = [x1_pool.tile([128, 512], F32R, tag=f"xc{k}", bufs=2,
                                       name=f"xc{tcb}_{k}")
                          for k in range(NKT)]
                    for k in range(NKT):
                        if tcb == 0:
                            nc.sync.dma_start(out=wqk[k][:],
                                              in_=wqkvT_d[k * 128:(k + 1) * 128,
                                                          0:2 * FH])
                        nc.sync.dma_start(
                            out=xc[k][:],
                            in_=xT_d[k * 128:(k + 1) * 128,
                                     tcb * 512:(tcb + 1) * 512])
                    if tcb == 1:
                        # prefetch v weights + proj weights during stage 1
                        for k in range(NKT):
                            nc.sync.dma_start(
                                out=wv[k][:],
                                in_=wqkvT_d[k * 128:(k + 1) * 128,
                                            2 * FH:3 * FH])
                        for g in range(4):
                            nc.sync.dma_start(
                                out=wprojT[g][:],
                                in_=wprojT_d[g * 128:(g + 1) * 128, :])
                    for fi in range(8):      # 0-3 q rows, 4-7 k rows
                        ps = ps1.tile([128, 512], F32, tag="ps1", bufs=6,
                                      name=f"psqk{tcb}_{fi}")
                        for k in range(NKT):
                            nc.tensor.matmul(ps[:],
                                             wqk[k][:, fi * 128:(fi + 1) * 128],
                                             xc[k][:],
                                             start=(k == 0), stop=(k == NKT - 1))
                        nc.vector.tensor_copy(
                            out=qkT[fi][:, tcb * 512:(tcb + 1) * 512], in_=ps[:])

            # ------- stage 2: v projection + attention + output proj -------
            with (
                tc.tile_pool(name="xc2", bufs=1) as x2_pool,
                tc.tile_pool(name="attnt", bufs=1) as attnt_pool,
                tc.tile_pool(name="pt", bufs=1) as pt_pool,
                tc.tile_pool(name="stage", bufs=1) as stage_pool,
                tc.tile_pool(name="nrm", bufs=1) as nrm_pool,
                tc.tile_pool(name="psM", bufs=1, space="PSUM") as psM,
                tc.tile_pool(name="psS", bufs=1, space="PSUM") as psS,
                tc.tile_pool(name="psPV", bufs=1, space="PSUM") as psPV,
            ):
                attnT = [attnt_pool.tile([128, T], F32R, tag=f"at{g}",
                                         name=f"at{g}") for g in range(4)]

                def emit_proj(ti, fc):
                    po = psM.tile([128, 512], F32, tag="mm512", bufs=2,
                                  name=f"po{ti}_{fc}")
                    for g in range(4):
                        nc.tensor.matmul(
                            po[:],
                            attnT[g][:, ti * 128:(ti + 1) * 128],
                            wprojT[g][:, fc * 512:(fc + 1) * 512],
                            start=(g == 0), stop=(g == 3))
                    ot = stage_pool.tile([128, 512], F32, tag="ot",
                                         bufs=2, name=f"ot{ti}_{fc}")
                    nc.vector.tensor_copy(out=ot[:], in_=po[:])
                    nc.sync.dma_start(
                        out=y_d[ti * 128:(ti + 1) * 128,
                                fc * 512:(fc + 1) * 512],
                        in_=ot[:])

                def emit_vgroup(ti):
                    xc = [x2_pool.tile([128, 128], F32R, tag=f"x2_{k}",
                                       bufs=2, name=f"x2_{ti}_{k}")
                          for k in range(NKT)]
                    for k in range(NKT):
                        nc.sync.dma_start(
                            out=xc[k][:],
                            in_=xT_d[k * 128:(k + 1) * 128,
                                     ti * 128:(ti + 1) * 128])
                    ps = psM.tile([128, 512], F32, tag="mm512", bufs=2,
                                  name=f"psv{ti}")
                    for k in range(NKT):
                        nc.tensor.matmul(ps[:],
                                         xc[k][:],
                                         wv[k][:],
                                         start=(k == 0), stop=(k == NKT - 1))
                    vt = v_sb[ti]
                    vv = vt[:].rearrange("p (h x) -> p h x", h=HPC)
                    nc.vector.memset(vt[:].bitcast(F32), 1.0)
                    nc.vector.tensor_copy(
                        out=vv[:, :, 0:64],
                        in_=ps[:].rearrange("p (h x) -> p h x", h=HPC))

                pending = []
                for tcb in range(NTC):
                    if tcb == 0:
                        for tl in range(4):
                            emit_vgroup(tl)

                    # ---- attention for i-block bi = tcb ----
                    bi = tcb
                    njt = 4 * bi + 4
                    for hp in range(4):          # head pair (2hp, 2hp+1)
                        # v projection of the NEXT t-chunk as PE filler
                        if tcb < NTC - 1:
                            emit_vgroup((tcb + 1) * 4 + hp)
                        for _ in range(2):
                            if pending:
                                emit_proj(*pending.pop(0))
                        qt = qkT[hp]
                        kt = qkT[4 + hp]
                        pts = []
                        pvs = [psPV.tile([65, 512], F32, tag="pv", bufs=2,
                                         name=f"pv{bi}_{hp}_{par}")
                               for par in range(2)]

                        def emit_scores(jj, bi=bi, hp=hp, qt=qt, kt=kt,
                                        pts=pts):
                            sps = psS.tile([128, 1024], F32, tag="sps", bufs=2,
                                           name=f"sps{bi}_{hp}_{jj}")
                            # even head on PE rows 0-63, odd head on rows
                            # 64-127 -> the two MMs run concurrently
                            for par in range(2):
                                off = par * 64
                                nc.tensor.matmul(
                                    sps[:, par * 512:(par + 1) * 512],
                                    kt[off:off + 64, jj * 128:(jj + 1) * 128],
                                    qt[off:off + 64, bi * 512:(bi + 1) * 512],
                                    start=True, stop=True)
                            pt = pt_pool.tile([128, 1024], F32R, tag="pt",
                                              bufs=LAG + 2,
                                              name=f"pt{bi}_{hp}_{jj}")
                            if jj < 4 * bi:
                                nc.scalar.activation(pt[:], sps[:], EXP,
                                                     scale=0.125)
                            else:
                                st = stage_pool.tile([128, 1024], F32R,
                                                     tag="st", bufs=2,
                                                     name=f"st{bi}_{hp}_{jj}")
                                r0 = jj - 4 * bi
                                lo = 128 * r0
                                sv = sps[:].rearrange("p (b i) -> p b i", b=2)
                                tv = st[:].rearrange("p (b i) -> p b i", b=2)
                                nc.scalar.activation(tv[:, :, lo:512],
                                                     sv[:, :, lo:512], EXP,
                                                     scale=0.125)
                                nc.gpsimd.affine_select(
                                    out=pt[:].rearrange("p (b i) -> p b i", b=2),
                                    in_=st[:].rearrange("p (b i) -> p b i", b=2),
                                    compare_op=mybir.AluOpType.is_ge,
                                    fill=0.0,
                                    base=-128 * r0,
                                    pattern=[[0, 2], [1, 512]],
                                    channel_multiplier=-1,
                                )
                            pts.append(pt)

                        def emit_pv(jj, bi=bi, hp=hp, pts=pts, pvs=pvs,
                                    njt=njt):
                            for par in range(2):
                                h = 2 * hp + par
                                nc.tensor.matmul(
                                    pvs[par][:],
                                    v_sb[jj][:, h * 65:h * 65 + 65],
                                    pts[jj][:, par * 512:(par + 1) * 512],
                                    start=(jj == 0), stop=(jj == njt - 1))

                        # software pipeline: PV lags scores by LAG j-tiles
                        for jj in range(njt):
                            emit_scores(jj)
                            if jj >= LAG:
                                emit_pv(jj - LAG)
                        for jj in range(max(0, njt - LAG), njt):
                            emit_pv(jj)
                        for par in range(2):
                            pv = pvs[par]
                            den = nrm_pool.tile([1, 512], F32, tag="den",
                                                bufs=1, name=f"den{bi}_{hp}_{par}")
                            nc.vector.tensor_copy(out=den[0:1, :],
                                                  in_=pv[64:65, :])
                            rec = nrm_pool.tile([1, 512], F32, tag="rec",
                                                bufs=1, name=f"rec{bi}_{hp}_{par}")
                            nc.vector.reciprocal_approx_fast(
                                out=rec[0:1, :], in_=den[0:1, :])
                            bc = nrm_pool.tile([64, 512], F32, tag="bc",
                                               bufs=2, name=f"bc{bi}_{hp}_{par}")
                            nc.gpsimd.partition_broadcast(bc[:, :], rec[0:1, :])
                            nc.vector.tensor_mul(
                                out=attnT[hp][par * 64:par * 64 + 64,
                                              bi * 512:(bi + 1) * 512],
                                in0=pv[0:64, :], in1=bc[:, :])
                    # queue this i-block's projection groups
                    for tl in range(4):
                        for fc in range(2):
                            pending.append((bi * 4 + tl, fc))
                for pf in pending:
                    emit_proj(*pf)
            wv_pool_outer.__exit__(None, None, None)
    nc.compile()
    return nc


def _get_nc():
    if "nc" not in _CACHE:
        _CACHE["nc"] = build_nc()
    return _CACHE["nc"]


def kernel(x, w_qkv, w_proj, _trace=False):
    x = np.asarray(x, dtype=np.float32)
    w_qkv = np.asarray(w_qkv, dtype=np.float32)
    w_proj = np.asarray(w_proj, dtype=np.float32)

    nc = _get_nc()
    in_maps = []
    for c in range(NCORES):
        hg, b = c // 4, c % 4
        xT = np.ascontiguousarray(x[b].T)                       # [1024, 2048]
        rows = []
        for sec in range(3):                                     # q, k, v
            rows.append(w_qkv[sec * C + hg * FH: sec * C + (hg + 1) * FH])
        wqkvT = np.ascontiguousarray(np.concatenate(rows, 0).T)  # [1024, 1536]
        wprojT = np.ascontiguousarray(w_proj[:, hg * FH:(hg + 1) * FH].T)
        in_maps.append({"xT": xT, "wqkvT": wqkvT, "wprojT": wprojT})

    res = run_bass_kernel_spmd(nc, in_maps, list(range(NCORES)), trace=_trace)
    if _trace:
        _CACHE["exec_time_ns"] = res.exec_time_ns

    y = np.empty((B, T, C), dtype=np.float32)
    for b in range(B):
        y[b] = res.results[b]["y"] + res.results[4 + b]["y"]
    return y


# revision 17
# speedup vs baseline: 1.0268x; 1.0262x over previous
"""Causal self-attention TRN2 kernel (8 NeuronCores).

Problem: x[4,2048,1024] f32, w_qkv[3072,1024], w_proj[1024,1024]
  qkv = x @ w_qkv.T; per-head causal softmax(q k^T / sqrt(64)) v; out @ w_proj.T

Sharding: 8 cores = (head-group hg in {0,1}) x (batch b in {0..3}).
  Core computes its 8 heads for its batch; partial y (contracted over its
  512 channels of w_proj input dim) is summed pairwise on host.

Per-core dataflow (all matmul inputs float32r = full-rate TF32-like):
  Stage 1: q,k projections -> qkT [1024, 2048] (f on partitions).
  Stage 2 (fused, per t-chunk): v projection -> V [2048, 8x65] (natural;
    col 65h+64 = ones giving the softmax denominator for free in PV),
    then attention for that i-block:
      S^T tiles [j=128, i=2x512] via PE (K=64, even/odd head row-tiled so
      the two MMs run concurrently), exp via ACT (scale=1/8) psum->sbuf,
      causal mask on straddling tiles via gpsimd.affine_select,
      PV via PE lhsT=[V|1] -> psum [65,512] (row 64 = denominator),
      normalize: DVE recip-approx + gpsimd partition_broadcast + DVE mul
      -> attnT [512, 2048]  (c_local on partitions)
    projection (attnT.T @ w_projT) interleaved lazily as PE filler.
"""

import numpy as np

import concourse.bacc as bacc
import concourse.mybir as mybir
import concourse.tile as tile
from concourse.bass_utils import run_bass_kernel_spmd

F32 = mybir.dt.float32
F32R = mybir.dt.float32r
EXP = mybir.ActivationFunctionType.Exp

B, T, C = 4, 2048, 1024
NH, HD = 16, 64
HPC = 8                      # heads per core
FH = HPC * HD                # 512: per-core q/k/v feature width
NCORES = 8
LAG = 2                      # scores->PV software-pipeline depth (j-tiles)

_CACHE = {}


def build_nc():
    nc = bacc.Bacc()
    xT_d = nc.dram_tensor("xT", [C, T], F32R, kind="ExternalInput")
    wqkvT_d = nc.dram_tensor("wqkvT", [C, 3 * FH], F32R, kind="ExternalInput")
    wprojT_d = nc.dram_tensor("wprojT", [FH, C], F32R, kind="ExternalInput")
    y_d = nc.dram_tensor("y", [T, C], F32, kind="ExternalOutput")

    NKT = C // 128           # 8 c-tiles (contraction for qkv)
    NTT = T // 128           # 16 t-tiles
    NTC = T // 512           # 4 t-chunks / i-blocks

    with tile.TileContext(nc) as tc:
        with (
            # ---------------- persistent pools (whole kernel) --------------
            tc.tile_pool(name="qkt", bufs=1) as qkt_pool,
            tc.tile_pool(name="vp", bufs=1) as v_pool,
            tc.tile_pool(name="wproj", bufs=1) as wproj_pool,
        ):
            qkT = [qkt_pool.tile([128, T], F32R, tag=f"qkt{i}", name=f"qkt{i}")
                   for i in range(8)]
            v_sb = [v_pool.tile([128, HPC * 65], F32R, tag=f"v{i}", name=f"v{i}")
                    for i in range(NTT)]
            wprojT = wproj_pool.tile([128, 4 * C], F32R, tag="wp", name="wp")

            wv_pool_outer = tc.tile_pool(name="wv", bufs=1)
            wv_pool = wv_pool_outer.__enter__()
            wv = wv_pool.tile([128, NKT * FH], F32R, tag="wv", name="wv")

            # ---------------- stage 1: q,k projections ---------------------
            with (
                tc.tile_pool(name="wqk", bufs=1) as wqk_pool,
                tc.tile_pool(name="xc1", bufs=1) as x1_pool,
                tc.tile_pool(name="ps1", bufs=1, space="PSUM") as ps1,
            ):
                wqk = wqk_pool.tile([128, NKT * 2 * FH], F32R, tag="wqk",
                                    name="wqk")
                nc.sync.dma_start(
                    out=wqk[:].rearrange("p (k f) -> p k f", k=NKT),
                    in_=wqkvT_d[0:C, 0:2 * FH].rearrange(
                        "(k p) f -> p k f", p=128))
                for tcb in range(NTC):
                    xc = x1_pool.tile([128, NKT * 512], F32R, tag="xc", bufs=2,
                                      name=f"xc{tcb}")
                    nc.sync.dma_start(
                        out=xc[:].rearrange("p (k t) -> p k t", k=NKT),
                        in_=xT_d[0:C, tcb * 512:(tcb + 1) * 512].rearrange(
                            "(k p) t -> p k t", p=128))
                    if tcb == 1:
                        # prefetch v weights + proj weights during stage 1
                        nc.sync.dma_start(
                            out=wv[:].rearrange("p (k f) -> p k f", k=NKT),
                            in_=wqkvT_d[0:C, 2 * FH:3 * FH].rearrange(
                                "(k p) f -> p k f", p=128))
                        nc.sync.dma_start(
                            out=wprojT[:].rearrange("p (g f) -> p g f", g=4),
                            in_=wprojT_d[0:FH, :].rearrange(
                                "(g p) f -> p g f", p=128))
                    for fi in range(8):      # 0-3 q rows, 4-7 k rows
                        ps = ps1.tile([128, 512], F32, tag="ps1", bufs=6,
                                      name=f"psqk{tcb}_{fi}")
                        for k in range(NKT):
                            nc.tensor.matmul(
                                ps[:],
                                wqk[:, k * 1024 + fi * 128:
                                    k * 1024 + (fi + 1) * 128],
                                xc[:, k * 512:(k + 1) * 512],
                                start=(k == 0), stop=(k == NKT - 1))
                        nc.vector.tensor_copy(
                            out=qkT[fi][:, tcb * 512:(tcb + 1) * 512], in_=ps[:])

            # ------- stage 2: v projection + attention + output proj -------
            with (
                tc.tile_pool(name="xc2", bufs=1) as x2_pool,
                tc.tile_pool(name="attnt", bufs=1) as attnt_pool,
                tc.tile_pool(name="pt", bufs=1) as pt_pool,
                tc.tile_pool(name="stage", bufs=1) as stage_pool,
                tc.tile_pool(name="nrm", bufs=1) as nrm_pool,
                tc.tile_pool(name="psM", bufs=1, space="PSUM") as psM,
                tc.tile_pool(name="psS", bufs=1, space="PSUM") as psS,
                tc.tile_pool(name="psPV", bufs=1, space="PSUM") as psPV,
            ):
                attnT = [attnt_pool.tile([128, T], F32R, tag=f"at{g}",
                                         name=f"at{g}") for g in range(4)]

                def emit_proj(ti, fc):
                    po = psM.tile([128, 512], F32, tag="mm512", bufs=2,
                                  name=f"po{ti}_{fc}")
                    for g in range(4):
                        nc.tensor.matmul(
                            po[:],
                            attnT[g][:, ti * 128:(ti + 1) * 128],
                            wprojT[:, g * C + fc * 512:
                                   g * C + (fc + 1) * 512],
                            start=(g == 0), stop=(g == 3))
                    ot = stage_pool.tile([128, 512], F32, tag="ot",
                                         bufs=2, name=f"ot{ti}_{fc}")
                    nc.vector.tensor_copy(out=ot[:], in_=po[:])
                    nc.sync.dma_start(
                        out=y_d[ti * 128:(ti + 1) * 128,
                                fc * 512:(fc + 1) * 512],
                        in_=ot[:])

                def emit_vgroup(ti):
                    xc = x2_pool.tile([128, NKT * 128], F32R, tag="x2",
                                      bufs=2, name=f"x2_{ti}")
                    nc.sync.dma_start(
                        out=xc[:].rearrange("p (k t) -> p k t", k=NKT),
                        in_=xT_d[0:C, ti * 128:(ti + 1) * 128].rearrange(
                            "(k p) t -> p k t", p=128))
                    ps = psM.tile([128, 512], F32, tag="mm512", bufs=2,
                                  name=f"psv{ti}")
                    for k in range(NKT):
                        nc.tensor.matmul(ps[:],
                                         xc[:, k * 128:(k + 1) * 128],
                                         wv[:, k * FH:(k + 1) * FH],
                                         start=(k == 0), stop=(k == NKT - 1))
                    vt = v_sb[ti]
                    vv = vt[:].rearrange("p (h x) -> p h x", h=HPC)
                    nc.vector.memset(vt[:].bitcast(F32), 1.0)
                    nc.vector.tensor_copy(
                        out=vv[:, :, 0:64],
                        in_=ps[:].rearrange("p (h x) -> p h x", h=HPC))

                pending = []
                for tcb in range(NTC):
                    if tcb == 0:
                        for tl in range(4):
                            emit_vgroup(tl)

                    # ---- attention for i-block bi = tcb ----
                    bi = tcb
                    njt = 4 * bi + 4
                    for hp in range(4):          # head pair (2hp, 2hp+1)
                        # v projection of the NEXT t-chunk as PE filler
                        if tcb < NTC - 1:
                            emit_vgroup((tcb + 1) * 4 + hp)
                        for _ in range(2):
                            if pending:
                                emit_proj(*pending.pop(0))
                        qt = qkT[hp]
                        kt = qkT[4 + hp]
                        pts = []
                        pvs = [psPV.tile([65, 512], F32, tag="pv", bufs=2,
                                         name=f"pv{bi}_{hp}_{par}")
                               for par in range(2)]

                        def emit_scores(jj, bi=bi, hp=hp, qt=qt, kt=kt,
                                        pts=pts):
                            sps = psS.tile([128, 1024], F32, tag="sps", bufs=2,
                                           name=f"sps{bi}_{hp}_{jj}")
                            # even head on PE rows 0-63, odd head on rows
                            # 64-127 -> the two MMs run concurrently
                            for par in range(2):
                                off = par * 64
                                nc.tensor.matmul(
                                    sps[:, par * 512:(par + 1) * 512],
                                    kt[off:off + 64, jj * 128:(jj + 1) * 128],
                                    qt[off:off + 64, bi * 512:(bi + 1) * 512],
                                    start=True, stop=True)
                            pt = pt_pool.tile([128, 1024], F32R, tag="pt",
                                              bufs=LAG + 2,
                                              name=f"pt{bi}_{hp}_{jj}")
                            if jj < 4 * bi:
                                nc.scalar.activation(pt[:], sps[:], EXP,
                                                     scale=0.125)
                            else:
                                st = stage_pool.tile([128, 1024], F32R,
                                                     tag="st", bufs=2,
                                                     name=f"st{bi}_{hp}_{jj}")
                                r0 = jj - 4 * bi
                                lo = 128 * r0
                                sv = sps[:].rearrange("p (b i) -> p b i", b=2)
                                tv = st[:].rearrange("p (b i) -> p b i", b=2)
                                nc.scalar.activation(tv[:, :, lo:512],
                                                     sv[:, :, lo:512], EXP,
                                                     scale=0.125)
                                nc.gpsimd.affine_select(
                                    out=pt[:].rearrange("p (b i) -> p b i", b=2),
                                    in_=st[:].rearrange("p (b i) -> p b i", b=2),
                                    compare_op=mybir.AluOpType.is_ge,
                                    fill=0.0,
                                    base=-128 * r0,
                                    pattern=[[0, 2], [1, 512]],
                                    channel_multiplier=-1,
                                )
                            pts.append(pt)

                        def emit_pv(jj, bi=bi, hp=hp, pts=pts, pvs=pvs,
                                    njt=njt):
                            for par in range(2):
                                h = 2 * hp + par
                                nc.tensor.matmul(
                                    pvs[par][:],
                                    v_sb[jj][:, h * 65:h * 65 + 65],
                                    pts[jj][:, par * 512:(par + 1) * 512],
                                    start=(jj == 0), stop=(jj == njt - 1))

                        # software pipeline: PV lags scores by LAG j-tiles
                        for jj in range(njt):
                            emit_scores(jj)
                            if jj >= LAG:
                                emit_pv(jj - LAG)
                        for jj in range(max(0, njt - LAG), njt):
                            emit_pv(jj)
                        for par in range(2):
                            pv = pvs[par]
                            den = nrm_pool.tile([1, 512], F32, tag="den",
                                                bufs=1, name=f"den{bi}_{hp}_{par}")
                            nc.vector.tensor_copy(out=den[0:1, :],
                                                  in_=pv[64:65, :])
                            rec = nrm_pool.tile([1, 512], F32, tag="rec",
                                                bufs=1, name=f"rec{bi}_{hp}_{par}")
                            nc.vector.reciprocal_approx_fast(
                                out=rec[0:1, :], in_=den[0:1, :])
                            bc = nrm_pool.tile([64, 512], F32, tag="bc",
                                               bufs=2, name=f"bc{bi}_{hp}_{par}")
                            nc.gpsimd.partition_broadcast(bc[:, :], rec[0:1, :])
                            nc.vector.tensor_mul(
                                out=attnT[hp][par * 64:par * 64 + 64,
                                              bi * 512:(bi + 1) * 512],
                                in0=pv[0:64, :], in1=bc[:, :])
                    # queue this i-block's projection groups
                    for tl in range(4):
                        for fc in range(2):
                            pending.append((bi * 4 + tl, fc))
                for pf in pending:
                    emit_proj(*pf)
            wv_pool_outer.__exit__(None, None, None)
    nc.compile()
    return nc


def _get_nc():
    if "nc" not in _CACHE:
        _CACHE["nc"] = build_nc()
    return _CACHE["nc"]


def kernel(x, w_qkv, w_proj, _trace=False):
    x = np.asarray(x, dtype=np.float32)
    w_qkv = np.asarray(w_qkv, dtype=np.float32)
    w_proj = np.asarray(w_proj, dtype=np.float32)

    nc = _get_nc()
    in_maps = []
    for c in range(NCORES):
        hg, b = c // 4, c % 4
        xT = np.ascontiguousarray(x[b].T)                       # [1024, 2048]
        rows = []
        for sec in range(3):                                     # q, k, v
            rows.append(w_qkv[sec * C + hg * FH: sec * C + (hg + 1) * FH])
        wqkvT = np.ascontiguousarray(np.concatenate(rows, 0).T)  # [1024, 1536]
        wprojT = np.ascontiguousarray(w_proj[:, hg * FH:(hg + 1) * FH].T)
        in_maps.append({"xT": xT, "wqkvT": wqkvT, "wprojT": wprojT})

    res = run_bass_kernel_spmd(nc, in_maps, list(range(NCORES)), trace=_trace)
    if _trace:
        _CACHE["exec_time_ns"] = res.exec_time_ns

    y = np.empty((B, T, C), dtype=np.float32)
    for b in range(B):
        y[b] = res.results[b]["y"] + res.results[4 + b]["y"]
    return y


# revision 19
# speedup vs baseline: 1.0432x; 1.0160x over previous
"""Causal self-attention TRN2 kernel (8 NeuronCores).

Problem: x[4,2048,1024] f32, w_qkv[3072,1024], w_proj[1024,1024]
  qkv = x @ w_qkv.T; per-head causal softmax(q k^T / sqrt(64)) v; out @ w_proj.T

Sharding: 8 cores = (head-group hg in {0,1}) x (batch b in {0..3}).
  Core computes its 8 heads for its batch; partial y (contracted over its
  512 channels of w_proj input dim) is summed pairwise on host.

Per-core dataflow (all matmul inputs float32r = full-rate TF32-like):
  Stage 1: q,k projections -> qkT [1024, 2048] (f on partitions).
  Stage 2 (fused, per t-chunk): v projection -> V [2048, 8x65] (natural;
    col 65h+64 = ones giving the softmax denominator for free in PV),
    then attention for that i-block:
      S^T tiles [j=128, i=2x512] via PE (K=64, even/odd head row-tiled so
      the two MMs run concurrently), exp via ACT (scale=1/8) psum->sbuf,
      causal mask on straddling tiles via gpsimd.affine_select,
      PV via PE lhsT=[V|1] -> psum [65,512] (row 64 = denominator),
      normalize: DVE recip-approx + gpsimd partition_broadcast + DVE mul
      -> attnT [512, 2048]  (c_local on partitions)
    projection (attnT.T @ w_projT) interleaved lazily as PE filler.
"""

import numpy as np

import concourse.bacc as bacc
import concourse.mybir as mybir
import concourse.tile as tile
from concourse.bass_utils import run_bass_kernel_spmd

F32 = mybir.dt.float32
F32R = mybir.dt.float32r
EXP = mybir.ActivationFunctionType.Exp

B, T, C = 4, 2048, 1024
NH, HD = 16, 64
HPC = 8                      # heads per core
FH = HPC * HD                # 512: per-core q/k/v feature width
NCORES = 8
LAG = 2                      # scores->PV software-pipeline depth (j-tiles)

_CACHE = {}


def build_nc():
    nc = bacc.Bacc()
    xT_d = nc.dram_tensor("xT", [C, T], F32R, kind="ExternalInput")
    wqkvT_d = nc.dram_tensor("wqkvT", [C, 3 * FH], F32R, kind="ExternalInput")
    wprojT_d = nc.dram_tensor("wprojT", [FH, C], F32R, kind="ExternalInput")
    y_d = nc.dram_tensor("y", [T, C], F32, kind="ExternalOutput")

    NKT = C // 128           # 8 c-tiles (contraction for qkv)
    NTT = T // 128           # 16 t-tiles
    NTC = T // 512           # 4 t-chunks / i-blocks

    with tile.TileContext(nc) as tc:
        with (
            # ---------------- persistent pools (whole kernel) --------------
            tc.tile_pool(name="qkt", bufs=1) as qkt_pool,
            tc.tile_pool(name="vp", bufs=1) as v_pool,
            tc.tile_pool(name="wproj", bufs=1) as wproj_pool,
        ):
            qkT = [qkt_pool.tile([128, T], F32R, tag=f"qkt{i}", name=f"qkt{i}")
                   for i in range(8)]
            v_sb = [v_pool.tile([128, HPC * 65], F32R, tag=f"v{i}", name=f"v{i}")
                    for i in range(NTT)]
            wprojT = wproj_pool.tile([128, 4 * C], F32R, tag="wp", name="wp")

            wv_pool_outer = tc.tile_pool(name="wv", bufs=1)
            wv_pool = wv_pool_outer.__enter__()
            wv = wv_pool.tile([128, NKT * FH], F32R, tag="wv", name="wv")

            psum_outer = [tc.tile_pool(name="psM", bufs=1, space="PSUM"),
                          tc.tile_pool(name="psS", bufs=1, space="PSUM"),
                          tc.tile_pool(name="psPV", bufs=1, space="PSUM")]
            psM = psum_outer[0].__enter__()
            psS = psum_outer[1].__enter__()
            psPV = psum_outer[2].__enter__()

            # ---------------- stage 1: q,k projections ---------------------
            with (
                tc.tile_pool(name="wqk", bufs=1) as wqk_pool,
                tc.tile_pool(name="xc1", bufs=1) as x1_pool,
            ):
                wqk = wqk_pool.tile([128, NKT * 2 * FH], F32R, tag="wqk",
                                    name="wqk")
                nc.sync.dma_start(
                    out=wqk[:].rearrange("p (k f) -> p k f", k=NKT),
                    in_=wqkvT_d[0:C, 0:2 * FH].rearrange(
                        "(k p) f -> p k f", p=128))
                for tcb in range(NTC):
                    xc = x1_pool.tile([128, NKT * 512], F32R, tag="xc", bufs=2,
                                      name=f"xc{tcb}")
                    nc.sync.dma_start(
                        out=xc[:].rearrange("p (k t) -> p k t", k=NKT),
                        in_=xT_d[0:C, tcb * 512:(tcb + 1) * 512].rearrange(
                            "(k p) t -> p k t", p=128))
                    if tcb == 1:
                        # prefetch v weights + proj weights during stage 1
                        nc.sync.dma_start(
                            out=wv[:].rearrange("p (k f) -> p k f", k=NKT),
                            in_=wqkvT_d[0:C, 2 * FH:3 * FH].rearrange(
                                "(k p) f -> p k f", p=128))
                        nc.sync.dma_start(
                            out=wprojT[:].rearrange("p (g f) -> p g f", g=4),
                            in_=wprojT_d[0:FH, :].rearrange(
                                "(g p) f -> p g f", p=128))
                    for fi in range(8):      # 0-3 q rows, 4-7 k rows
                        ps = psM.tile([128, 512], F32, tag="mm512", bufs=2,
                                      name=f"psqk{tcb}_{fi}")
                        for k in range(NKT):
                            nc.tensor.matmul(
                                ps[:],
                                wqk[:, k * 1024 + fi * 128:
                                    k * 1024 + (fi + 1) * 128],
                                xc[:, k * 512:(k + 1) * 512],
                                start=(k == 0), stop=(k == NKT - 1))
                        nc.vector.tensor_copy(
                            out=qkT[fi][:, tcb * 512:(tcb + 1) * 512], in_=ps[:])

            # ------- stage 2: v projection + attention + output proj -------
            with (
                tc.tile_pool(name="xc2", bufs=1) as x2_pool,
                tc.tile_pool(name="attnt", bufs=1) as attnt_pool,
                tc.tile_pool(name="pt", bufs=1) as pt_pool,
                tc.tile_pool(name="stage", bufs=1) as stage_pool,
                tc.tile_pool(name="nrm", bufs=1) as nrm_pool,
            ):
                attnT = [attnt_pool.tile([128, T], F32R, tag=f"at{g}",
                                         name=f"at{g}") for g in range(4)]

                def emit_proj(ti, fc):
                    po = psM.tile([128, 512], F32, tag="mm512", bufs=2,
                                  name=f"po{ti}_{fc}")
                    for g in range(4):
                        nc.tensor.matmul(
                            po[:],
                            attnT[g][:, ti * 128:(ti + 1) * 128],
                            wprojT[:, g * C + fc * 512:
                                   g * C + (fc + 1) * 512],
                            start=(g == 0), stop=(g == 3))
                    ot = stage_pool.tile([128, 512], F32, tag="ot",
                                         bufs=2, name=f"ot{ti}_{fc}")
                    nc.vector.tensor_copy(out=ot[:], in_=po[:])
                    nc.sync.dma_start(
                        out=y_d[ti * 128:(ti + 1) * 128,
                                fc * 512:(fc + 1) * 512],
                        in_=ot[:])

                def emit_vgroup(ti):
                    xc = x2_pool.tile([128, NKT * 128], F32R, tag="x2",
                                      bufs=2, name=f"x2_{ti}")
                    nc.sync.dma_start(
                        out=xc[:].rearrange("p (k t) -> p k t", k=NKT),
                        in_=xT_d[0:C, ti * 128:(ti + 1) * 128].rearrange(
                            "(k p) t -> p k t", p=128))
                    ps = psM.tile([128, 512], F32, tag="mm512", bufs=2,
                                  name=f"psv{ti}")
                    for k in range(NKT):
                        nc.tensor.matmul(ps[:],
                                         xc[:, k * 128:(k + 1) * 128],
                                         wv[:, k * FH:(k + 1) * FH],
                                         start=(k == 0), stop=(k == NKT - 1))
                    vt = v_sb[ti]
                    vv = vt[:].rearrange("p (h x) -> p h x", h=HPC)
                    nc.vector.memset(vt[:].bitcast(F32), 1.0)
                    nc.vector.tensor_copy(
                        out=vv[:, :, 0:64],
                        in_=ps[:].rearrange("p (h x) -> p h x", h=HPC))

                pending = []
                for tcb in range(NTC):
                    if tcb == 0:
                        for tl in range(4):
                            emit_vgroup(tl)

                    # ---- attention for i-block bi = tcb ----
                    bi = tcb
                    njt = 4 * bi + 4
                    for hp in range(4):          # head pair (2hp, 2hp+1)
                        # v projection of the NEXT t-chunk as PE filler
                        if tcb < NTC - 1:
                            emit_vgroup((tcb + 1) * 4 + hp)
                        for _ in range(2):
                            if pending:
                                emit_proj(*pending.pop(0))
                        qt = qkT[hp]
                        kt = qkT[4 + hp]
                        pts = []
                        pvs = [psPV.tile([65, 512], F32, tag="pv", bufs=2,
                                         name=f"pv{bi}_{hp}_{par}")
                               for par in range(2)]

                        def emit_scores(jj, bi=bi, hp=hp, qt=qt, kt=kt,
                                        pts=pts):
                            sps = psS.tile([128, 1024], F32, tag="sps", bufs=2,
                                           name=f"sps{bi}_{hp}_{jj}")
                            # even head on PE rows 0-63, odd head on rows
                            # 64-127 -> the two MMs run concurrently
                            for par in range(2):
                                off = par * 64
                                nc.tensor.matmul(
                                    sps[:, par * 512:(par + 1) * 512],
                                    kt[off:off + 64, jj * 128:(jj + 1) * 128],
                                    qt[off:off + 64, bi * 512:(bi + 1) * 512],
                                    start=True, stop=True)
                            pt = pt_pool.tile([128, 1024], F32R, tag="pt",
                                              bufs=LAG + 2,
                                              name=f"pt{bi}_{hp}_{jj}")
                            if jj < 4 * bi:
                                nc.scalar.activation(pt[:], sps[:], EXP,
                                                     scale=0.125)
                            else:
                                st = stage_pool.tile([128, 1024], F32R,
                                                     tag="st", bufs=2,
                                                     name=f"st{bi}_{hp}_{jj}")
                                r0 = jj - 4 * bi
                                lo = 128 * r0
                                sv = sps[:].rearrange("p (b i) -> p b i", b=2)
                                tv = st[:].rearrange("p (b i) -> p b i", b=2)
                                nc.scalar.activation(tv[:, :, lo:512],
                                                     sv[:, :, lo:512], EXP,
                                                     scale=0.125)
                                nc.gpsimd.affine_select(
                                    out=pt[:].rearrange("p (b i) -> p b i", b=2),
                                    in_=st[:].rearrange("p (b i) -> p b i", b=2),
                                    compare_op=mybir.AluOpType.is_ge,
                                    fill=0.0,
                                    base=-128 * r0,
                                    pattern=[[0, 2], [1, 512]],
                                    channel_multiplier=-1,
                                )
                            pts.append(pt)

                        def emit_pv(jj, bi=bi, hp=hp, pts=pts, pvs=pvs,
                                    njt=njt):
                            for par in range(2):
                                h = 2 * hp + par
                                nc.tensor.matmul(
                                    pvs[par][:],
                                    v_sb[jj][:, h * 65:h * 65 + 65],
                                    pts[jj][:, par * 512:(par + 1) * 512],
                                    start=(jj == 0), stop=(jj == njt - 1))

                        # software pipeline: PV lags scores by LAG j-tiles
                        for jj in range(njt):
                            emit_scores(jj)
                            if jj >= LAG:
                                emit_pv(jj - LAG)
                        for jj in range(max(0, njt - LAG), njt):
                            emit_pv(jj)
                        for par in range(2):
                            pv = pvs[par]
                            den = nrm_pool.tile([1, 512], F32, tag="den",
                                                bufs=1, name=f"den{bi}_{hp}_{par}")
                            nc.vector.tensor_copy(out=den[0:1, :],
                                                  in_=pv[64:65, :])
                            rec = nrm_pool.tile([1, 512], F32, tag="rec",
                                                bufs=1, name=f"rec{bi}_{hp}_{par}")
                            nc.vector.reciprocal_approx_fast(
                                out=rec[0:1, :], in_=den[0:1, :])
                            bc = nrm_pool.tile([64, 512], F32, tag="bc",
                                               bufs=2, name=f"bc{bi}_{hp}_{par}")
                            nc.gpsimd.partition_broadcast(bc[:, :], rec[0:1, :])
                            nc.vector.tensor_mul(
                                out=attnT[hp][par * 64:par * 64 + 64,
                                              bi * 512:(bi + 1) * 512],
                                in0=pv[0:64, :], in1=bc[:, :])
                    # queue this i-block's projection groups
                    for tl in range(4):
                        for fc in range(2):
                            pending.append((bi * 4 + tl, fc))
                for pf in pending:
                    emit_proj(*pf)
            for p in reversed(psum_outer):
                p.__exit__(None, None, None)
            wv_pool_outer.__exit__(None, None, None)
    nc.compile()
    return nc


def _get_nc():
    if "nc" not in _CACHE:
        _CACHE["nc"] = build_nc()
    return _CACHE["nc"]


def kernel(x, w_qkv, w_proj, _trace=False):
    x = np.asarray(x, dtype=np.float32)
    w_qkv = np.asarray(w_qkv, dtype=np.float32)
    w_proj = np.asarray(w_proj, dtype=np.float32)

    nc = _get_nc()
    in_maps = []
    for c in range(NCORES):
        hg, b = c // 4, c % 4
        xT = np.ascontiguousarray(x[b].T)                       # [1024, 2048]
        rows = []
        for sec in range(3):                                     # q, k, v
            rows.append(w_qkv[sec * C + hg * FH: sec * C + (hg + 1) * FH])
        wqkvT = np.ascontiguousarray(np.concatenate(rows, 0).T)  # [1024, 1536]
        wprojT = np.ascontiguousarray(w_proj[:, hg * FH:(hg + 1) * FH].T)
        in_maps.append({"xT": xT, "wqkvT": wqkvT, "wprojT": wprojT})

    res = run_bass_kernel_spmd(nc, in_maps, list(range(NCORES)), trace=_trace)
    if _trace:
        _CACHE["exec_time_ns"] = res.exec_time_ns

    y = np.empty((B, T, C), dtype=np.float32)
    for b in range(B):
        y[b] = res.results[b]["y"] + res.results[4 + b]["y"]
    return y
